# revision 21
# baseline (speedup 1.0000x reference)
"""Chamfer distance kernel for Trainium2 (8 NeuronCores).

Problem: xyz1, xyz2 [B=4, N=M=8192, 3] f32.
  d[b,n,m] = ||x1-x2||^2 ; outputs dist1/idx1 = min/argmin over m,
  dist2/idx2 = min/argmin over n.

Strategy per core (8 cores = 4 batches x 2 halves):
  core c handles batch b=c//2, half h=c%2.
  Pass A: rows = xyz1[b, h*4096:(h+1)*4096], cols = all xyz2[b]  -> dist1/idx1 half
  Pass B: rows = xyz2[b, h*4096:(h+1)*4096], cols = all xyz1[b]  -> dist2/idx2 half

On-chip, per 128-row block:
  - PE computes full distance tiles via one fused K=24 bf16-split matmul
    (see KDIM comment): psum[p, f] = d(row p, col f) to ~f32 accuracy
  - ACT copies PSUM -> SBUF X (source for the argmin gather)
  - DVE tensor_reduce (3D view) computes WIN-wide window minima TMF
  - v = min(TMF) is the distance output; winning window s* found via
    is_equal + (eq * descending-iota, max-reduce)
  - GPSIMD ap_gather fetches each partition's winning WIN-window (indices
    are shared per 16-partition group, so 15/16 of the gathered slots are
    other partitions' windows of the same row; the CG constant masks all
    foreign slots to zero, which also makes tie-breaking exactly
    "first index" like the reference)
  - final index = WIN*s* + r*; the G-chain is emitted one block late so
    the in-order DVE queue never stalls on the Pool gather
"""

import sys

sys.path.insert(0, "/opt/trn_rl_repo")

import numpy as np

import concourse.bacc as bacc
import concourse.mybir as mybir
from concourse.tile import TileContext
from concourse.bass_utils import run_bass_kernel_spmd

F32 = mybir.dt.float32
BF16 = mybir.dt.bfloat16
I32 = mybir.dt.int32
I16 = mybir.dt.int16
AX = mybir.AxisListType.X
OP = mybir.AluOpType

# bf16-split matmul: each f32 operand is split exactly into h+m+l bf16
# parts; per coordinate we keep the 6 largest cross products (hh, hm, mh,
# mm, hl, lh; the dropped ml/lm/ll are < 2^-24 relative).  sq1/sq2 ride in
# as 3 split rows each against exact 'ones'.  All products are exact in
# f32; only the PSUM f32 accumulation rounds, so the distance matches a
# straight f32 computation to ~1-2 ulp.
KDIM = 24
WIN = 16          # argmin gather window width
GENW = 2048       # psum generation width (4 banks)


def build_nc(nblk, m, n_cores=8, stages=3, repeat=1):
    """nblk: number of 128-row blocks per pass; m: rhs width (cols).

    stages (debug): 1 = matmul+reduce+v only, 2 = +F chain (no gather),
    3 = full.  repeat (debug): run the whole body N times (for slope
    timing)."""
    nrow = nblk * 128
    ngen = m // GENW
    nwin = m // WIN            # windows per block row
    wpg = GENW // WIN          # windows per generation

    nc = bacc.Bacc("TRN2", target_bir_lowering=False, debug=False,
                   num_devices=n_cores)

    la_d = nc.dram_tensor("la", [KDIM, nrow], BF16, kind="ExternalInput")
    ra_d = nc.dram_tensor("ra", [KDIM, m], BF16, kind="ExternalInput")
    lb_d = nc.dram_tensor("lb", [KDIM, nrow], BF16, kind="ExternalInput")
    rb_d = nc.dram_tensor("rb", [KDIM, m], BF16, kind="ExternalInput")
    cf_d = nc.dram_tensor("cf", [128, nwin], F32, kind="ExternalInput")
    cg_d = nc.dram_tensor("cg", [128, 16 * WIN], F32, kind="ExternalInput")

    da_d = nc.dram_tensor("da", [128, nblk], F32, kind="ExternalOutput")
    ia_d = nc.dram_tensor("ia", [128, nblk], I32, kind="ExternalOutput")
    db_d = nc.dram_tensor("db", [128, nblk], F32, kind="ExternalOutput")
    ib_d = nc.dram_tensor("ib", [128, nblk], I32, kind="ExternalOutput")

    with TileContext(nc) as tc:
        with (
            tc.tile_pool(name="const", bufs=1) as cpool,
            tc.tile_pool(name="psum", bufs=2, space="PSUM") as ppool,
            tc.tile_pool(name="x", bufs=3) as xpool,
            tc.tile_pool(name="tmf", bufs=3) as tmfpool,
            tc.tile_pool(name="scr", bufs=3) as spool,
            tc.tile_pool(name="small", bufs=6) as mpool,
            tc.tile_pool(name="acc", bufs=1) as apool,
        ):
            LA = cpool.tile([KDIM, nrow], BF16, tag="la")
            RA = cpool.tile([KDIM, m], BF16, tag="ra")
            LB = cpool.tile([KDIM, nrow], BF16, tag="lb")
            RB = cpool.tile([KDIM, m], BF16, tag="rb")
            CF = cpool.tile([128, nwin], F32, tag="cf")
            CG = cpool.tile([128, 16 * WIN], F32, tag="cg")
            nc.sync.dma_start(LA[:], la_d[:])
            nc.sync.dma_start(RA[:], ra_d[:])
            nc.sync.dma_start(LB[:], lb_d[:])
            nc.sync.dma_start(RB[:], rb_d[:])
            nc.sync.dma_start(CF[:], cf_d[:])
            nc.sync.dma_start(CG[:], cg_d[:])

            pass_cfgs = [
                (LA, RA, da_d, ia_d, "a"),
                (LB, RB, db_d, ib_d, "b"),
            ] * repeat
            def back_chain(st):
                # deferred argmin extraction (one block behind, so the DVE
                # queue never head-of-line blocks on the Pool gather)
                v, sstar, G, iacc, b = st
                eqG = spool.tile([128, 16 * WIN], F32, tag="eqg")
                nc.vector.tensor_scalar(eqG[:], G[:], v, None, op0=OP.is_equal)
                scrG = spool.tile([128, 16 * WIN], F32, tag="scrg")
                rd = mpool.tile([128, 1], F32, tag="rd")
                nc.vector.tensor_mul(scrG[:], eqG[:], CG[:])
                nc.vector.tensor_reduce(rd[:], scrG[:], axis=AX, op=OP.max)
                # idx = WIN*s* + r* = WIN*s* + (WIN - rd)
                t2 = mpool.tile([128, 1], F32, tag="t2")
                nc.vector.tensor_scalar(t2[:], rd[:], -1.0, float(WIN),
                                        op0=OP.mult, op1=OP.add)
                t3 = mpool.tile([128, 1], F32, tag="t3")
                nc.vector.tensor_scalar(t3[:], sstar[:], float(WIN), None,
                                        op0=OP.mult)
                nc.vector.tensor_add(iacc[:, b:b + 1], t3[:], t2[:])

            pending = None
            for pi, (L, R, d_out, i_out, acctag) in enumerate(pass_cfgs):
                dacc = apool.tile([128, nblk], F32, tag=f"dacc{acctag}")
                iacc = apool.tile([128, nblk], F32, tag=f"iacc{acctag}")
                for b in range(nblk):
                    X = xpool.tile([128, m], F32, tag="x")
                    TMF = tmfpool.tile([128, nwin], F32, tag="tmf")
                    lslice = L[:, b * 128:(b + 1) * 128]
                    for g in range(ngen):
                        ps = ppool.tile([128, GENW], F32, tag="ps")
                        for q in range(GENW // 512):
                            nc.tensor.matmul(
                                ps[:, q * 512:(q + 1) * 512],
                                lslice,
                                R[:, g * GENW + q * 512: g * GENW + (q + 1) * 512],
                                start=True, stop=True,
                            )
                        nc.scalar.copy(X[:, g * GENW:(g + 1) * GENW], ps[:])
                        nc.vector.tensor_reduce(
                            TMF[:, g * wpg:(g + 1) * wpg],
                            ps.rearrange("p (s r) -> p s r", r=WIN),
                            axis=AX, op=OP.min,
                        )
                    # v (the min distance) straight into the output accumulator
                    v = dacc[:, b:b + 1]
                    nc.vector.tensor_reduce(v, TMF[:], axis=AX, op=OP.min)
                    if stages < 2:
                        nc.vector.tensor_copy(iacc[:, b:b + 1], v)
                        continue
                    # winning window s*: eqF = (TMF == v); sd = max(eqF * (nwin - s))
                    eqF = spool.tile([128, nwin], F32, tag="eqf")
                    nc.vector.tensor_scalar(eqF[:], TMF[:], v, None, op0=OP.is_equal)
                    scrF = spool.tile([128, nwin], F32, tag="scrf")
                    sd = mpool.tile([128, 1], F32, tag="sd")
                    nc.vector.tensor_mul(scrF[:], eqF[:], CF[:])
                    nc.vector.tensor_reduce(sd[:], scrF[:], axis=AX, op=OP.max)
                    # winning window index s* = nwin - sd
                    sstar = mpool.tile([128, 1], F32, tag="sstar")
                    nc.vector.tensor_scalar(sstar[:], sd[:], -1.0, float(nwin),
                                            op0=OP.mult, op1=OP.add)
                    if stages < 3:
                        nc.vector.tensor_copy(iacc[:, b:b + 1], sstar)
                        continue
                    gidx16 = mpool.tile([128, 1], I16, tag="gidx16")
                    nc.vector.tensor_copy(gidx16[:], sstar[:])
                    G = spool.tile([128, 16 * WIN], F32, tag="g")
                    nc.gpsimd.ap_gather(
                        G.rearrange("p (i r) -> p i r", r=WIN),
                        X.rearrange("p (s r) -> p s r", r=WIN),
                        gidx16[:],
                        channels=128, num_elems=nwin, d=WIN, num_idxs=16,
                    )
                    if pending is not None:
                        back_chain(pending)
                    pending = (v, sstar, G, iacc, b)
                if pending is not None:
                    back_chain(pending)
                    pending = None
                ii = apool.tile([128, nblk], I32, tag=f"ii{acctag}")
                nc.vector.tensor_copy(ii[:], iacc[:])
                nc.sync.dma_start(d_out[:], dacc[:])
                nc.sync.dma_start(i_out[:], ii[:])

    nc.compile()
    return nc


def _const_cf(nwin):
    # descending window iota: value nwin - s at window s, replicated rows
    return np.broadcast_to(
        (nwin - np.arange(nwin, dtype=np.float32)), (128, nwin)).copy()


def _const_cg():
    # (WIN - r) in each partition's own gather slot (j == p % 16), else 0.
    # Masking foreign slots makes tie-breaking exactly "first index" and
    # removes any junk-window contamination.
    out = np.zeros((128, 16 * WIN), dtype=np.float32)
    r = np.arange(WIN, dtype=np.float32)
    for p in range(128):
        j = p % 16
        out[p, j * WIN:(j + 1) * WIN] = WIN - r
    return out


import ml_dtypes

BF = ml_dtypes.bfloat16


def _split3(x):
    """Exact 3-way bf16 split: x ~= h + m + l (residual < 2^-24 rel)."""
    x = x.astype(np.float32)
    h = x.astype(BF)
    r = x - h.astype(np.float32)
    m = r.astype(BF)
    r2 = r - m.astype(np.float32)
    l = r2.astype(BF)
    return h, m, l


def _prep_l(pts):
    """pts [n,3] f32 -> lhsT [24, n] bf16 (see KDIM comment)."""
    n = pts.shape[0]
    out = np.empty((KDIM, n), dtype=BF)
    a = (-2.0 * pts.T).astype(np.float32)  # exact power-of-two scale
    for c in range(3):
        ah, am, al = _split3(a[c])
        out[6 * c + 0] = ah
        out[6 * c + 1] = ah
        out[6 * c + 2] = am
        out[6 * c + 3] = am
        out[6 * c + 4] = ah
        out[6 * c + 5] = al
    one = np.ones((n,), dtype=BF)
    out[18] = one
    out[19] = one
    out[20] = one
    s1h, s1m, s1l = _split3((pts * pts).sum(axis=1, dtype=np.float32))
    out[21] = s1h
    out[22] = s1m
    out[23] = s1l
    return out


def _prep_r(pts):
    """pts [m,3] f32 -> rhs [24, m] bf16 (see KDIM comment)."""
    mm = pts.shape[0]
    out = np.empty((KDIM, mm), dtype=BF)
    b = pts.T.astype(np.float32)
    for c in range(3):
        bh, bm, bl = _split3(b[c])
        out[6 * c + 0] = bh
        out[6 * c + 1] = bm
        out[6 * c + 2] = bh
        out[6 * c + 3] = bm
        out[6 * c + 4] = bl
        out[6 * c + 5] = bh
    s2h, s2m, s2l = _split3((pts * pts).sum(axis=1, dtype=np.float32))
    out[18] = s2h
    out[19] = s2m
    out[20] = s2l
    one = np.ones((mm,), dtype=BF)
    out[21] = one
    out[22] = one
    out[23] = one
    return out


_NC_CACHE = {}


def _get_nc(nblk, m):
    key = (nblk, m)
    if key not in _NC_CACHE:
        _NC_CACHE[key] = build_nc(nblk, m)
    return _NC_CACHE[key]


def kernel(xyz1, xyz2):
    xyz1 = np.asarray(xyz1, dtype=np.float32)
    xyz2 = np.asarray(xyz2, dtype=np.float32)
    B, N, _ = xyz1.shape
    M = xyz2.shape[1]
    assert (B, N, M) == (4, 8192, 8192), (B, N, M)
    half = N // 2
    nblk = half // 128

    nc = _get_nc(nblk, M)
    cf = _const_cf(M // WIN)
    cg = _const_cg()

    in_maps = []
    for c in range(8):
        b, h = divmod(c, 2)
        in_maps.append({
            "la": _prep_l(xyz1[b, h * half:(h + 1) * half]),
            "ra": _prep_r(xyz2[b]),
            "lb": _prep_l(xyz2[b, h * half:(h + 1) * half]),
            "rb": _prep_r(xyz1[b]),
            "cf": cf,
            "cg": cg,
        })

    res = run_bass_kernel_spmd(nc, in_maps, core_ids=list(range(8)))

    dist1 = np.empty((B, N), dtype=np.float32)
    idx1 = np.empty((B, N), dtype=np.int32)
    dist2 = np.empty((B, M), dtype=np.float32)
    idx2 = np.empty((B, M), dtype=np.int32)
    for c in range(8):
        b, h = divmod(c, 2)
        sl = slice(h * half, (h + 1) * half)
        r = res.results[c]
        dist1[b, sl] = r["da"].T.reshape(-1)
        idx1[b, sl] = r["ia"].T.reshape(-1)
        dist2[b, sl] = r["db"].T.reshape(-1)
        idx2[b, sl] = r["ib"].T.reshape(-1)
    return dist1, dist2, idx1, idx2
